# revision 21
# baseline (speedup 1.0000x reference)
"""CTC loss (Keras ctc_batch_cost semantics) on 8 Trainium2 NeuronCores.

Strategy: pure data parallelism — batch B=1024 sharded 128/core (batch =
SBUF partition dim). Host does index preparation only (extended-label
gather of y_pred, skip masks folded into a second prob tensor PM=P*mask,
reachability mask, seam/readout fold, s-axis reversal of the forward
lattice); each core runs the CTC lattice sum in the linear-probability
domain in bf16 on the Vector engine alone (Pool/GpSimd shares SBUF ports
with DVE and contended runs measured slower than single-engine). The
forward DP (t=0..127, s-reversed) and the backward suffix DP in gamma
form (G_t = p_t*B_t, t=255..128) are FUSED side by side in one
[128, 198] tile so each DP step is 4 double-pumped bf16 instructions:
  a = X + X>>1 ; b = X>>2 * PM_t ; c = a * P_t ; X' = c + b
(the >>k reads are +k column offsets; both chains shift the same way
because the forward lattice is reversed). Guard columns between/after
the halves are re-zeroed for free by zero pad columns in P/PM. The host
prescales every (b,t) row of P by a power of two so its max is ~1, which
bounds state drift to ~87 bits per 32 steps; every 32 steps each half is
renormalized to 2^120 by a power-of-2 scale from its row max, and the
host adds the exact prescale correction ln2*sum(k_t) back to the loss.
The halves meet in a one-time log-domain seam (per-lane exact bit logs
+ one reversed copy + logsumexp — lane products span ~2^-175 and would
underflow linear f32):
ll = m + ln Σ_s exp(lnA+lnB−m) + ln2·(Σ shifts − 2·120 − 3·127).
No collectives; host concatenates the per-core [128,1] outputs.
"""

import numpy as np
import ml_dtypes

import concourse.bacc as bacc
import concourse.mybir as mybir
import concourse.tile as tile
from concourse.bass_utils import run_bass_kernel_spmd

B, T, C, U = 1024, 256, 100, 48
S = 2 * U + 1          # 97 extended-label positions
HW = S + 2             # left half incl 2 guard cols (fwd starts at col 0)
W2 = 2 * HW            # 198: [fwd 0:97 | g | bwd 99:196 | g]
BLANK = C - 1
EPS = 1e-7
NCORES = 8
BS = B // NCORES       # 128 samples per core = SBUF partition dim
CH = 32                # time steps per DMA chunk
TH = 128               # fused steps + init slot
RENORM = 32            # host prescale bounds drift to ~87 bits/32 steps
RT_LOG2 = 120          # renorm target 2^120 (also the host init scale)
EVENTS = [i for i in range(2, TH - 1) if i % RENORM == 0]  # 32,64,96
NEV = len(EVENTS)
NLG = 2 * NEV
F32 = mybir.dt.float32
BF16 = mybir.dt.bfloat16
I32 = mybir.dt.int32
ALU = mybir.AluOpType
AXX = mybir.AxisListType.X
ACTF = mybir.ActivationFunctionType
LN2 = float(np.log(2.0))
TINY = 1e-38
DEAD = -1000.0


def _emit(nc, tc, p_d, pm_d, out_d):
    v = nc.vector
    with tc.tile_pool(name="pchunks", bufs=1) as ppool, tc.tile_pool(
        name="work", bufs=1
    ) as wp:
        ps = []
        pms = []
        for ci in range(TH // CH):
            pt_ = ppool.tile([BS, CH * W2], BF16, name=f"p{ci}")
            pmt = ppool.tile([BS, CH * W2], BF16, name=f"pm{ci}")
            if ci == 0:
                # split the first chunks across queues: the first DP step
                # waits on them, later chunks hide behind compute.
                for j in range(16):
                    sl = slice(j * 2 * W2, (j + 1) * 2 * W2)
                    nc.sync.dma_start(out=pt_[:, sl], in_=p_d[:, j * 2 : (j + 1) * 2, :])
                    nc.sync.dma_start(out=pmt[:, sl], in_=pm_d[:, j * 2 : (j + 1) * 2, :])
            else:
                nc.sync.dma_start(out=pt_[:], in_=p_d[:, ci * CH : (ci + 1) * CH, :])
                nc.sync.dma_start(out=pmt[:], in_=pm_d[:, ci * CH : (ci + 1) * CH, :])
            ps.append(pt_)
            pms.append(pmt)

        xA = wp.tile([BS, W2], BF16, name="xA")
        xB = wp.tile([BS, W2], BF16, name="xB")
        a_t = wp.tile([BS, W2 - 2], BF16, name="a_t")
        b_t = wp.tile([BS, W2 - 2], BF16, name="b_t")
        c_t = wp.tile([BS, W2 - 2], BF16, name="c_t")
        mx2 = wp.tile([BS, 2], F32, name="mx2")
        k2 = wp.tile([BS, 2], I32, name="k2")
        sc2 = wp.tile([BS, 2], I32, name="sc2")
        lgi = wp.tile([BS, NLG], I32, name="lgi")
        lgall = wp.tile([BS, NLG], F32, name="lgall")
        d_in = wp.tile([BS, 1], F32, name="d_in")
        d_out = wp.tile([BS, 1], F32, name="d_out")

        v.memset(xA[:], 0.0)
        v.memset(xB[:], 0.0)
        v.memset(lgi[:], 0)
        # preload the ACT Ln table during the DMA wait (first seam use
        # would otherwise serialize a ~1.3us table load; Exp still loads
        # once in the seam — preloading it here would evict Ln).
        v.memset(d_in[:], 1.0)
        nc.scalar.activation(out=d_out[:], in_=d_in[:], func=ACTF.Ln)
        # init row (host-built): [rev alpha_0 | sel*p_255], both * 2^RT.
        v.tensor_copy(out=xA[:, 0:W2], in_=ps[0][:, 0:W2])

        lg_col = {i: 2 * e for e, i in enumerate(EVENTS)}

        for i in range(1, TH):
            ci, off = i // CH, (i % CH) * W2
            pt = ps[ci][:, off : off + W2 - 2]
            pmt = pms[ci][:, off : off + W2 - 2]
            cur, nxt = (xA, xB) if i % 2 == 1 else (xB, xA)
            ev = i in EVENTS
            ap = i % RENORM == 1 and i > 1

            if ap:
                # apply last event's per-half scales to the carried state
                # (plain dual-op TS: STT measures ~250ns with no 2x mode)
                v.tensor_scalar(out=cur[:, 0:HW], in0=cur[:, 0:HW], scalar1=sc2[:].bitcast(F32)[:, 0:1], scalar2=None, op0=ALU.mult)
                v.tensor_scalar(out=cur[:, HW:W2], in0=cur[:, HW:W2], scalar1=sc2[:].bitcast(F32)[:, 1:2], scalar2=None, op0=ALU.mult)
            v.tensor_tensor(out=a_t[:], in0=cur[:, 0 : W2 - 2], in1=cur[:, 1 : W2 - 1], op=ALU.add)
            v.tensor_tensor(out=b_t[:], in0=cur[:, 2:W2], in1=pmt, op=ALU.mult)
            v.tensor_tensor(out=c_t[:], in0=a_t[:], in1=pt, op=ALU.mult)
            v.tensor_tensor(out=nxt[:, 0 : W2 - 2], in0=c_t[:], in1=b_t[:], op=ALU.add)

            if ev:
                c = lg_col[i]
                v.tensor_reduce(out=mx2[:, 0:1], in_=nxt[:, 0:S], axis=AXX, op=ALU.max)
                v.tensor_reduce(out=mx2[:, 1:2], in_=nxt[:, HW : HW + S], axis=AXX, op=ALU.max)
                # k = clamp(e(max)-RT, -126) into lgi; sc = bits of 2^-k
                v.tensor_scalar(out=k2[:], in0=mx2[:].bitcast(I32), scalar1=23, scalar2=None, op0=ALU.logical_shift_right)
                v.tensor_scalar(out=lgi[:, c : c + 2], in0=k2[:], scalar1=127 + RT_LOG2, scalar2=-126, op0=ALU.subtract, op1=ALU.max)
                v.tensor_scalar(out=sc2[:], in0=lgi[:, c : c + 2], scalar1=-1, scalar2=127, op0=ALU.mult, op1=ALU.add)
                v.tensor_scalar(out=sc2[:], in0=sc2[:], scalar1=23, scalar2=None, op0=ALU.logical_shift_left)

        # TH-1=127 odd => final fused state lives in xB.
        xfin = xB

        # ---- seam partial: B_127 = G + G^(+1) + skip2*G^(+2), computed
        # with the same fused ops; PM slot 0 carries [0 | skip2].
        pm0 = pms[0][:, 0 : W2 - 2]
        v.tensor_tensor(out=a_t[:], in0=xfin[:, 0 : W2 - 2], in1=xfin[:, 1 : W2 - 1], op=ALU.add)
        v.tensor_tensor(out=b_t[:], in0=xfin[:, 2:W2], in1=pm0, op=ALU.mult)
        v.tensor_tensor(out=c_t[:], in0=a_t[:], in1=b_t[:], op=ALU.add)

        # ---- log-domain seam ----
        ff32 = wp.tile([BS, S], F32, name="ff32")
        bf32 = wp.tile([BS, S], F32, name="bf32")
        as_t = wp.tile([BS, S], F32, name="as_t")
        ei_t = wp.tile([BS, S], I32, name="ei_t")
        mi_t = wp.tile([BS, S], I32, name="mi_t")
        ef_t = wp.tile([BS, S], F32, name="ef_t")
        lm_t = wp.tile([BS, S], F32, name="lm_t")
        la_t = wp.tile([BS, S], F32, name="la_t")
        da_t = wp.tile([BS, S], F32, name="da_t")
        la2 = wp.tile([BS, S], F32, name="la2")
        la2r = wp.tile([BS, S], F32, name="la2r")
        lb2 = wp.tile([BS, S], F32, name="lb2")
        lam = wp.tile([BS, S], F32, name="lam")
        m_t = wp.tile([BS, 1], F32, name="m_t")
        nm_t = wp.tile([BS, 1], F32, name="nm_t")
        e_t = wp.tile([BS, S], F32, name="e_t")
        z_t = wp.tile([BS, 1], F32, name="z_t")
        lnz_t = wp.tile([BS, 1], F32, name="lnz_t")
        racc = wp.tile([BS, 1], F32, name="racc")
        acc1 = wp.tile([BS, 1], F32, name="acc1")
        acc2 = wp.tile([BS, 1], F32, name="acc2")
        outt = wp.tile([BS, 1], F32, name="outt")

        def side_log(x32, out_lam):
            # exact log of f32 x: split exponent/mantissa so the Ln LUT only
            # sees [1,2); zero lanes forced to DEAD. The biased-exponent
            # -127*ln2 per side is folded into the final constant.
            v.tensor_scalar(out=as_t[:], in0=x32[:], scalar1=1.0, scalar2=TINY, op0=ALU.mult, op1=ALU.add)
            ai = as_t[:].bitcast(I32)
            v.tensor_scalar(out=ei_t[:], in0=ai, scalar1=23, scalar2=None, op0=ALU.logical_shift_right)
            v.tensor_scalar(out=mi_t[:], in0=ai, scalar1=0x007FFFFF, scalar2=0x3F800000, op0=ALU.bitwise_and, op1=ALU.bitwise_or)
            v.tensor_copy(out=ef_t[:], in_=ei_t[:])
            nc.scalar.activation(out=lm_t[:], in_=mi_t[:].bitcast(F32), func=ACTF.Ln)
            v.scalar_tensor_tensor(out=la_t[:], in0=ef_t[:], scalar=LN2, in1=lm_t[:], op0=ALU.mult, op1=ALU.add)
            v.tensor_scalar(out=da_t[:], in0=x32[:], scalar1=0.0, scalar2=None, op0=ALU.is_equal)
            v.scalar_tensor_tensor(out=out_lam[:], in0=da_t[:], scalar=DEAD, in1=la_t[:], op0=ALU.mult, op1=ALU.add)

        v.tensor_copy(out=ff32[:], in_=xfin[:, 0:S])
        side_log(ff32, la2)
        v.tensor_copy(out=bf32[:], in_=c_t[:, HW : HW + S])
        side_log(bf32, lb2)
        # un-reverse the fwd side so lanes line up with the bwd side
        v.tensor_copy(out=la2r[:, 0:S], in_=la2[:, S - 1 :: -1])
        v.tensor_tensor(out=lam[:], in0=la2r[:], in1=lb2[:], op=ALU.add)
        v.tensor_reduce(out=m_t[:], in_=lam[:], axis=AXX, op=ALU.max)
        v.tensor_scalar(out=nm_t[:], in0=m_t[:], scalar1=-1.0, scalar2=None, op0=ALU.mult)
        nc.scalar.activation(out=e_t[:], in_=lam[:], func=ACTF.Exp, bias=nm_t[:, 0:1], scale=1.0)
        v.tensor_reduce(out=z_t[:], in_=e_t[:], axis=AXX, op=ALU.add)
        # ln z on DVE (bit-split + deg-2 poly on the mantissa, ~6e-3 abs —
        # trivial against the tolerance) instead of a third ACT Ln, which
        # would reload the Ln LUT (~1.3us) evicted by Exp.
        ezi = wp.tile([BS, 1], I32, name="ezi")
        ezf = wp.tile([BS, 1], F32, name="ezf")
        mzi = wp.tile([BS, 1], I32, name="mzi")
        pz1 = wp.tile([BS, 1], F32, name="pz1")
        pz2 = wp.tile([BS, 1], F32, name="pz2")
        zi = z_t[:].bitcast(I32)
        v.tensor_scalar(out=ezi[:], in0=zi, scalar1=23, scalar2=None, op0=ALU.logical_shift_right)
        v.tensor_scalar(out=mzi[:], in0=zi, scalar1=0x007FFFFF, scalar2=0x3F800000, op0=ALU.bitwise_and, op1=ALU.bitwise_or)
        v.tensor_copy(out=ezf[:], in_=ezi[:])
        mzf = mzi[:].bitcast(F32)
        v.tensor_scalar(out=pz1[:], in0=mzf, scalar1=-0.23350870, scalar2=1.38276158, op0=ALU.mult, op1=ALU.add)
        v.tensor_tensor(out=pz2[:], in0=pz1[:], in1=mzf, op=ALU.mult)
        v.tensor_scalar(out=pz2[:], in0=pz2[:], scalar1=-1.14299441, scalar2=None, op0=ALU.add)
        v.scalar_tensor_tensor(out=lnz_t[:], in0=ezf[:], scalar=LN2, in1=pz2[:], op0=ALU.mult, op1=ALU.add)
        v.tensor_copy(out=lgall[:], in_=lgi[:])
        v.tensor_reduce(out=racc[:], in_=lgall[:], axis=AXX, op=ALU.add)
        # constants: -2*RT for the host 2^RT injections, -127 for each of
        # the two side_log biased exponents and the ln z exponent.
        v.tensor_scalar(out=racc[:], in0=racc[:], scalar1=float(-2 * RT_LOG2 - 254 - 127), scalar2=None, op0=ALU.add)
        v.scalar_tensor_tensor(out=acc1[:], in0=racc[:], scalar=LN2, in1=m_t[:], op0=ALU.mult, op1=ALU.add)
        v.tensor_tensor(out=acc2[:], in0=acc1[:], in1=lnz_t[:], op=ALU.add)
        v.tensor_scalar(out=outt[:], in0=acc2[:], scalar1=-1.0, scalar2=None, op0=ALU.mult)
        nc.sync.dma_start(out=out_d, in_=outt[:])


def _build_program():
    nc = bacc.Bacc("TRN2", target_bir_lowering=False, debug=False)
    p_d = nc.dram_tensor("p", [BS, TH, W2], BF16, kind="ExternalInput").ap()
    pm_d = nc.dram_tensor("pm", [BS, TH, W2], BF16, kind="ExternalInput").ap()
    out_d = nc.dram_tensor("out", [BS, 1], F32, kind="ExternalOutput").ap()
    with tile.TileContext(nc) as tc:
        _emit(nc, tc, p_d, pm_d, out_d)
    nc.compile()
    return nc


_NC = None


def _get_nc():
    global _NC
    if _NC is None:
        _NC = _build_program()
    return _NC


def _prep_in_maps(y_pred, y_true, label_length):
    ext = np.full((B, S), BLANK, np.int32)
    ext[:, 1::2] = y_true.astype(np.int32)
    prev2 = np.concatenate([np.full((B, 2), BLANK, np.int32), ext[:, :-2]], axis=1)
    skip = ((ext != BLANK) & (ext != prev2)).astype(np.float32)
    skip2 = np.concatenate([skip[:, 2:], np.zeros((B, 2), np.float32)], axis=1)
    P = np.take_along_axis(
        np.ascontiguousarray(y_pred, dtype=np.float32), ext[:, None, :], axis=2
    )
    P += np.float32(EPS)
    L = label_length.reshape(B).astype(np.int64)
    i2 = np.clip(2 * L, 0, S - 1)
    i1 = np.maximum(i2 - 1, 0)
    # reachability mask: position s at time t is dead if it cannot reach i1
    # by t=T-1 (max +2 per step). Folded into P at zero device cost.
    s_idx = np.arange(S)[None, None, :]
    t_idx = np.arange(T)[None, :, None]
    alive = (s_idx + 2 * (T - 1 - t_idx)) >= i1[:, None, None]
    P *= alive.astype(np.float32)
    # host prescale: scale each (b,t) row to max ~1 (exact powers of two);
    # the exact correction Sum k_t * ln2 is added back on the host.
    k_t = np.round(np.log2(P.max(2))).astype(np.int32)
    P = P * np.exp2(-k_t.astype(np.float64))[:, :, None].astype(np.float32)
    kcorr = k_t.sum(1).astype(np.float64)
    sel = np.zeros((B, S), np.float32)
    sel[np.arange(B), i1] = 1.0
    sel[np.arange(B), i2] = 1.0
    SCALE = np.float32(2.0**RT_LOG2)
    # fused rows: left = s-reversed fwd probs, right = bwd probs (t falling)
    pfull = np.zeros((B, TH, W2), np.float32)
    pmfull = np.zeros((B, TH, W2), np.float32)
    pfull[:, 1:, 0:S] = P[:, 1:TH, ::-1]
    pfull[:, 1:, HW : HW + S] = P[:, T - 2 : T - 1 - TH : -1, :]
    skr = skip[:, ::-1]
    pmfull[:, 1:, 0:S] = pfull[:, 1:, 0:S] * skr[:, None, :]
    pmfull[:, 1:, HW : HW + S] = pfull[:, 1:, HW : HW + S] * skip2[:, None, :]
    # init row: [rev alpha_0 | sel*p_255] * 2^110
    pfull[:, 0, S - 2] = P[:, 0, 1] * SCALE
    pfull[:, 0, S - 1] = P[:, 0, 0] * SCALE
    pfull[:, 0, HW : HW + S] = sel * P[:, T - 1, :] * SCALE
    # PM slot 0 = seam mask [0 | skip2] for the final partial step
    pmfull[:, 0, HW : HW + S] = skip2
    pfull = pfull.astype(ml_dtypes.bfloat16)
    pmfull = pmfull.astype(ml_dtypes.bfloat16)
    in_maps = []
    for c in range(NCORES):
        sl = slice(c * BS, (c + 1) * BS)
        in_maps.append(
            {
                "p": np.ascontiguousarray(pfull[sl]),
                "pm": np.ascontiguousarray(pmfull[sl]),
            }
        )
    return in_maps, kcorr


def _run_device(in_maps, **kwargs):
    nc = _get_nc()
    return run_bass_kernel_spmd(nc, in_maps, core_ids=list(range(NCORES)), **kwargs)


def _ctc_numpy(y_pred, y_true, input_length, label_length):
    """Generality safety net (log domain, mirrors the reference exactly)."""
    b, t_max, c = y_pred.shape
    u = y_true.shape[1]
    s = 2 * u + 1
    blank = c - 1
    neg = np.float32(-1e30)
    logp = np.log(y_pred.astype(np.float32) + np.float32(EPS))
    ext = np.full((b, s), blank, np.int32)
    ext[:, 1::2] = y_true.astype(np.int32)
    prev2 = np.concatenate([np.full((b, 2), blank, np.int32), ext[:, :-2]], axis=1)
    can_skip = (ext != blank) & (ext != prev2)
    lp_ext = np.take_along_axis(logp, ext[:, None, :], axis=2)
    alpha = np.full((b, s), neg, np.float32)
    alpha[:, 0] = lp_ext[:, 0, 0]
    alpha[:, 1] = lp_ext[:, 0, 1]
    inp_len = input_length.reshape(b)

    def lse(stack):
        m = np.max(stack, axis=0)
        return m + np.log(np.sum(np.exp(stack - m), axis=0))

    for t in range(1, t_max):
        a1 = np.concatenate([np.full((b, 1), neg, np.float32), alpha[:, :-1]], axis=1)
        a2 = np.concatenate([np.full((b, 2), neg, np.float32), alpha[:, :-2]], axis=1)
        a2 = np.where(can_skip, a2, neg)
        new = lse(np.stack([alpha, a1, a2], 0)).astype(np.float32) + lp_ext[:, t, :]
        alpha = np.where((t < inp_len)[:, None], new, alpha)
    L = label_length.reshape(b).astype(np.int64)
    i2 = np.clip(2 * L, 0, s - 1)
    i1 = np.maximum(i2 - 1, 0)
    a_last = np.stack([alpha[np.arange(b), i1], alpha[np.arange(b), i2]], axis=1)
    ll = np.where(L > 0, lse(a_last.T).astype(np.float32), alpha[:, 0])
    return (-ll[:, None]).astype(np.float32)


def kernel(y_pred, y_true, input_length, label_length):
    y_pred = np.asarray(y_pred)
    y_true = np.asarray(y_true)
    input_length = np.asarray(input_length)
    label_length = np.asarray(label_length)
    if y_pred.shape != (B, T, C) or y_true.shape != (B, U) or not np.all(
        input_length.reshape(-1) == T
    ) or np.any(label_length.reshape(-1) <= 0):
        return _ctc_numpy(y_pred, y_true, input_length, label_length)
    in_maps, kcorr = _prep_in_maps(y_pred, y_true, label_length)
    res = _run_device(in_maps)
    out = np.concatenate([r["out"] for r in res.results], axis=0)
    out = out - (LN2 * kcorr)[:, None].astype(np.float32)
    return np.ascontiguousarray(out, dtype=np.float32)
